# revision 16
# baseline (speedup 1.0000x reference)
"""DDSP core synthesizer kernel for Trainium2 (8 NeuronCores, data-parallel).

Reference computation (per row of B=32, T=64000):
    f0_hz = 20*exp(f0); phase = cumsum(2*pi*f0_hz/SR)
    hw    = sum_k sin(phase*k)/k   (k = 1..60)
    audio = mix*hw*loud + (1-mix)*noise*loud;  out = audio / (max|audio| + 1e-6)

Device algorithm (phase in "turns"; harmonics in Q32 int fixed-point):
    inc  = exp(f0 + ln(20/SR))                       [ACT Exp]
    u    = blocked cumsum of inc                      [DVE scan + PE triangular mm]
    u1   = u - rint(u)  in [-0.5, 0.5]                [DVE magic rint + tt subtract]
    per harmonic k (flat layout: 128 blocks x 2000):
        v_k = k*u1*2^32 mod 2^32  (int32, exact wrap-around phase)
          k=1:    v_1 = int32(u1 * 2^32)              [DVE mult]
          even:   v_2m = v_m << 1                     [DVE arith_shift_left, wraps]
          odd:    w = int32(u1 * k*2^26); v = w << 6  [DVE mult + shift]
            or    v_k = v_{k-2} + v_2 (mod 2^32)      [GpSimd tt add, wraps]
        s_k = sin(2*pi*2^-32 * v_k)  -> f16           [ACT Sin, int32 input]
        hw += diag(1/k) @ s_k                         [PE f16 matmul, PSUM accum]
    epilogue: audio = Bm*hw + A with Bm = loud*mix, A = noise*(loud-Bm)
              peak-normalize per row (free abs-max + 32x32 transpose trick).

Sharding: pure data parallel, 4 rows per core, SPMD on cores 0-7.
"""

import sys

sys.path.insert(0, "/opt/trn_rl_repo")

import numpy as np
import ml_dtypes
from contextlib import ExitStack

import concourse.bass as bass
import concourse.tile as tile
from concourse import bacc, mybir
from concourse import bass_utils

f32 = np.float32
dt = mybir.dt

SR = 44100.0
H = 60
B, T = 32, 64000
NCORES = 8
RPC = B // NCORES           # rows per core = 4
P = 128
FD = T * RPC // P           # 2000
BPR = P // RPC              # 32
PI = float(np.pi)
MAGIC = float(1.5 * 2.0 ** 23)
EXP_BIAS = float(np.log(20.0 / SR))
Q26 = float(2.0 ** 26)
Q32 = float(2.0 ** 32)

_cache = {}


def _chains():
    order = []
    for h in range(1, H + 1, 2):
        k = h
        while k <= H:
            order.append(k)
            k *= 2
    return order


def _consts():
    kk, mm_ = np.meshgrid(np.arange(P), np.arange(P), indexing="ij")
    lt = ((kk // BPR == mm_ // BPR) & (kk % BPR < mm_ % BPR)).astype(f32)
    diag = np.zeros((H, P, P), dtype=np.float16)
    for k in range(1, H + 1):
        diag[k - 1, np.arange(P), np.arange(P)] = np.float16(1.0 / k)
    return {"lt": lt, "diag": diag}


def _build(gp_heads=19, vbufs=6, sbufs=5):
    nc = bacc.Bacc("TRN2", target_bir_lowering=False, debug=False,
                   enable_asserts=True, num_devices=NCORES)

    f0_d = nc.dram_tensor("f0", [P, FD], dt.float32, kind="ExternalInput")
    loud_d = nc.dram_tensor("loud", [P, FD], dt.float32, kind="ExternalInput")
    mix_d = nc.dram_tensor("mix", [P, FD], dt.float32, kind="ExternalInput")
    noise_d = nc.dram_tensor("noise", [P, FD], dt.float32, kind="ExternalInput")
    lt_d = nc.dram_tensor("lt", [P, P], dt.float32, kind="ExternalInput")
    diag_d = nc.dram_tensor("diag", [H, P, P], dt.float16, kind="ExternalInput")
    out_d = nc.dram_tensor("audio", [P, FD], dt.float32, kind="ExternalOutput")

    AF = mybir.ActivationFunctionType
    ALU = mybir.AluOpType

    gp_odd = set(range(3, 3 + 2 * gp_heads, 2)) & set(range(3, H, 2))

    with tile.TileContext(nc) as tc, ExitStack() as ctx:
        pool = ctx.enter_context(tc.tile_pool(name="sb", bufs=1))
        vpool = ctx.enter_context(tc.tile_pool(name="vp", bufs=vbufs))
        spool = ctx.enter_context(tc.tile_pool(name="sp", bufs=sbufs))
        xpool = ctx.enter_context(tc.tile_pool(name="xps", bufs=1, space="PSUM"))
        hpool = ctx.enter_context(tc.tile_pool(name="hps", bufs=1, space="PSUM"))

        def const_col(val, tag):
            t = pool.tile([P, 1], dt.float32, tag=tag)
            nc.vector.memset(t[:], val)
            return t

        exp_bias = const_col(EXP_BIAS, "cbias_exp")
        zero_bias = const_col(0.0, "cbias_zero")

        # ---- input DMA ----
        f0 = pool.tile([P, FD], dt.float32, tag="scr", bufs=4, name="f0")
        nc.sync.dma_start(f0[:], f0_d.ap())
        lt = pool.tile([P, P], dt.float32)
        nc.gpsimd.dma_start(lt[:], lt_d.ap())
        diag = pool.tile([P, H, P], dt.float16)
        nc.gpsimd.dma_start(diag[:], diag_d.ap().rearrange("g p m -> p g m"))
        loud = pool.tile([P, FD], dt.float32, tag="loud")
        nc.scalar.dma_start(loud[:], loud_d.ap())
        mix = pool.tile([P, FD], dt.float32, tag="mix")
        nc.scalar.dma_start(mix[:], mix_d.ap())
        noise = pool.tile([P, FD], dt.float32, tag="noise")
        nc.scalar.dma_start(noise[:], noise_d.ap())

        # ---- stage 1: phase accumulation (turns) ----
        inc = pool.tile([P, FD], dt.float32, tag="scr", bufs=4, name="inc")
        nc.scalar.activation(inc[:], f0[:], AF.Exp, bias=exp_bias[:, 0:1], scale=1.0)
        local = pool.tile([P, FD], dt.float32, tag="scr", bufs=4, name="local")
        nc.vector.tensor_tensor_scan(local[:], inc[:], inc[:], 0.0,
                                     ALU.add, ALU.bypass)
        offs_ps = xpool.tile([P, 1], dt.float32, tag="x")
        nc.tensor.matmul(offs_ps[:], lt[:], local[:, FD - 1:FD],
                         start=True, stop=True)
        offs = pool.tile([P, 1], dt.float32)
        nc.vector.tensor_copy(offs[:], offs_ps[:])
        u = pool.tile([P, FD], dt.float32, tag="scr", bufs=4, name="u")
        nc.vector.tensor_scalar(u[:], local[:], offs[:, 0:1], None, ALU.add)
        ur = pool.tile([P, FD], dt.float32, tag="scr", bufs=4, name="ur")
        nc.vector.tensor_scalar(ur[:], u[:], MAGIC, MAGIC, ALU.add, ALU.subtract)
        u1 = pool.tile([P, FD], dt.float32, tag="u1")
        nc.vector.tensor_tensor(u1[:], u[:], ur[:], ALU.subtract)

        # ---- stage 2: harmonic bank, flat layout ----
        hw_ps = hpool.tile([P, 4, 512], dt.float32, tag="hw")
        chunks = []
        c0 = 0
        while c0 < FD:
            cn = min(512, FD - c0)
            chunks.append((c0, cn))
            c0 += cn

        order = _chains()
        vmap = {}
        v2_res = pool.tile([P, FD], dt.int32, tag="v2res")
        prev_odd = [None]

        Bm = pool.tile([P, FD], dt.float32, tag="Bm")
        Am = pool.tile([P, FD], dt.float32, tag="Am")
        A = pool.tile([P, FD], dt.float32, tag="A")
        epi_at = {order[min(len(order) - 1, 8)]: 0}
        emitted_epi = [False]

        def emit_epi():
            nc.gpsimd.tensor_tensor(Bm[:], loud[:], mix[:], ALU.mult)
            nc.gpsimd.tensor_tensor(Am[:], loud[:], Bm[:], ALU.subtract)
            nc.gpsimd.tensor_tensor(A[:], noise[:], Am[:], ALU.mult)
            emitted_epi[0] = True

        first_k = order[0]
        last_k = order[-1]
        for ki, k in enumerate(order):
            if k in epi_at and not emitted_epi[0]:
                emit_epi()
            if k == 1:
                v = vpool.tile([P, FD], dt.int32, tag="v")
                nc.vector.tensor_scalar(v[:], u1[:], Q32, None, ALU.mult)
            elif k % 2 == 0:
                src = vmap[k // 2]
                if k == 2:
                    v = v2_res
                else:
                    v = vpool.tile([P, FD], dt.int32, tag="v")
                nc.vector.tensor_scalar(v[:], src[:], 1, None,
                                        ALU.arith_shift_left)
            elif k in gp_odd and prev_odd[0] is not None:
                v = vpool.tile([P, FD], dt.int32, tag="v")
                nc.gpsimd.tensor_tensor(v[:], prev_odd[0][:], v2_res[:], ALU.add)
            else:
                w = vpool.tile([P, FD], dt.int32, tag="v")
                nc.vector.tensor_scalar(w[:], u1[:], float(k) * Q26, None,
                                        ALU.mult)
                v = vpool.tile([P, FD], dt.int32, tag="v")
                nc.vector.tensor_scalar(v[:], w[:], 6, None,
                                        ALU.arith_shift_left)
            vmap[k] = v
            if k % 2 == 1:
                prev_odd[0] = v

            s = spool.tile([P, FD], dt.float16, tag="s")
            nc.scalar.activation(s[:], v[:], AF.Sin, bias=zero_bias[:, 0:1],
                                 scale=float(2.0 * PI / Q32))

            for q, (c0, cn) in enumerate(chunks):
                nc.tensor.matmul(hw_ps[:, q, 0:cn], diag[:, k - 1, :],
                                 s[:, c0:c0 + cn],
                                 start=(k == first_k), stop=(k == last_k))

        if not emitted_epi[0]:
            emit_epi()

        # ---- epilogue: audio = Bm*hw + A, then peak-normalize per row ----
        hw_flat = hw_ps[:].rearrange("p q f -> p (q f)")[:, 0:FD]
        t1 = pool.tile([P, FD], dt.float32, tag="t1")
        nc.vector.tensor_tensor(t1[:], hw_flat, Bm[:], ALU.mult)
        audio = pool.tile([P, FD], dt.float32, tag="audio")
        nc.vector.tensor_tensor(audio[:], t1[:], A[:], ALU.add)

        pk = pool.tile([P, 1], dt.float32, tag="pk")
        nc.vector.tensor_reduce(pk[:], audio[:], axis=mybir.AxisListType.X,
                                op=ALU.max, apply_absolute_value=True)
        pkr = pool.tile([P, 32], dt.float32, tag="pkr")
        nc.vector.tensor_copy(pkr[:], pk[:, 0:1].to_broadcast((P, 32)))
        pkt = pool.tile([P, 32], dt.float32, tag="pkt")
        nc.vector.transpose(pkt[:], pkr[:])
        rowmax = pool.tile([P, 1], dt.float32, tag="rowmax")
        nc.vector.tensor_reduce(rowmax[:], pkt[:],
                                axis=mybir.AxisListType.X, op=ALU.max)
        pke = pool.tile([P, 1], dt.float32, tag="pke")
        nc.vector.tensor_scalar(pke[:], rowmax[:], 1e-6, None, ALU.add)
        rcp = pool.tile([P, 1], dt.float32, tag="rcp")
        nc.vector.reciprocal(rcp[:], pke[:])
        outt = pool.tile([P, FD], dt.float32, tag="outt")
        nc.vector.tensor_scalar(outt[:], audio[:], rcp[:, 0:1], None, ALU.mult)
        nc.sync.dma_start(out_d.ap(), outt[:])

    nc.compile()
    return nc


def kernel(f0, loudness, harmonic_mix, noise):
    if "nc" not in _cache:
        _cache["nc"] = _build()
        _cache["consts"] = _consts()
    nc = _cache["nc"]
    consts = _cache["consts"]

    def shard(a, c):
        return np.ascontiguousarray(
            a[c * RPC:(c + 1) * RPC].astype(f32, copy=False).reshape(P, FD))

    in_maps = []
    for c in range(NCORES):
        in_maps.append({
            "f0": shard(f0, c),
            "loud": shard(loudness, c),
            "mix": shard(harmonic_mix, c),
            "noise": shard(noise, c),
            **consts,
        })

    res = bass_utils.run_bass_kernel_spmd(nc, in_maps, core_ids=list(range(NCORES)))
    outs = [res.results[c]["audio"].reshape(RPC, T) for c in range(NCORES)]
    return np.concatenate(outs, axis=0)


# revision 17
# speedup vs baseline: 1.0588x; 1.0588x over previous
"""DDSP core synthesizer kernel for Trainium2 (8 NeuronCores, data-parallel).

Reference computation (per row of B=32, T=64000):
    f0_hz = 20*exp(f0); phase = cumsum(2*pi*f0_hz/SR)
    hw    = sum_k sin(phase*k)/k   (k = 1..60)
    audio = mix*hw*loud + (1-mix)*noise*loud;  out = audio / (max|audio| + 1e-6)

Device algorithm (phase in "turns"; harmonics in Q32 int fixed-point):
    inc  = exp(f0 + ln(20/SR))                       [ACT Exp]
    u    = blocked cumsum of inc                      [DVE scan + PE triangular mm]
    u1   = u - rint(u)  in [-0.5, 0.5]                [DVE magic rint + tt subtract]
    per harmonic k (flat layout: 128 blocks x 2000):
        v_k = k*u1*2^32 mod 2^32  (int32, exact wrap-around phase)
          k=1:    v_1 = int32(u1 * 2^32)              [DVE mult]
          even:   v_2m = v_m << 1                     [DVE arith_shift_left, wraps]
          odd:    w = int32(u1 * k*2^26); v = w << 6  [DVE mult + shift]
            or    v_k = v_{k-2} + v_2 (mod 2^32)      [GpSimd tt add, wraps]
        s_k = sin(2*pi*2^-32 * v_k)  -> f16           [ACT Sin, int32 input]
        hw += diag(1/k) @ s_k                         [PE f16 matmul, PSUM accum]
    epilogue: audio = Bm*hw + A with Bm = loud*mix, A = noise*(loud-Bm)
              peak-normalize per row (free abs-max + 32x32 transpose trick).

Sharding: pure data parallel, 4 rows per core, SPMD on cores 0-7.
"""

import sys

sys.path.insert(0, "/opt/trn_rl_repo")

import numpy as np
import ml_dtypes
from contextlib import ExitStack

import concourse.bass as bass
import concourse.tile as tile
from concourse import bacc, mybir
from concourse import bass_utils

f32 = np.float32
dt = mybir.dt

SR = 44100.0
H = 60
B, T = 32, 64000
NCORES = 8
RPC = B // NCORES           # rows per core = 4
P = 128
FD = T * RPC // P           # 2000
BPR = P // RPC              # 32
PI = float(np.pi)
MAGIC = float(1.5 * 2.0 ** 23)
EXP_BIAS = float(np.log(20.0 / SR))
Q26 = float(2.0 ** 26)
Q32 = float(2.0 ** 32)

_cache = {}


def _chains():
    order = []
    for h in range(1, H + 1, 2):
        k = h
        while k <= H:
            order.append(k)
            k *= 2
    return order


def _consts():
    kk, mm_ = np.meshgrid(np.arange(P), np.arange(P), indexing="ij")
    lt = ((kk // BPR == mm_ // BPR) & (kk % BPR < mm_ % BPR)).astype(f32)
    diag = np.zeros((H, P, P), dtype=np.float16)
    for k in range(1, H + 1):
        diag[k - 1, np.arange(P), np.arange(P)] = np.float16(1.0 / k)
    return {"lt": lt, "diag": diag}


def _build(gp_heads=12, vbufs=6, sbufs=5):
    nc = bacc.Bacc("TRN2", target_bir_lowering=False, debug=False,
                   enable_asserts=True, num_devices=NCORES)

    f0_d = nc.dram_tensor("f0", [P, FD], dt.float32, kind="ExternalInput")
    loud_d = nc.dram_tensor("loud", [P, FD], dt.float32, kind="ExternalInput")
    mix_d = nc.dram_tensor("mix", [P, FD], dt.float32, kind="ExternalInput")
    noise_d = nc.dram_tensor("noise", [P, FD], dt.float32, kind="ExternalInput")
    lt_d = nc.dram_tensor("lt", [P, P], dt.float32, kind="ExternalInput")
    diag_d = nc.dram_tensor("diag", [H, P, P], dt.float16, kind="ExternalInput")
    out_d = nc.dram_tensor("audio", [P, FD], dt.float32, kind="ExternalOutput")

    AF = mybir.ActivationFunctionType
    ALU = mybir.AluOpType

    gp_odd = set(range(3, 3 + 2 * gp_heads, 2)) & set(range(3, H, 2))

    with tile.TileContext(nc) as tc, ExitStack() as ctx:
        pool = ctx.enter_context(tc.tile_pool(name="sb", bufs=1))
        vpool = ctx.enter_context(tc.tile_pool(name="vp", bufs=vbufs))
        spool = ctx.enter_context(tc.tile_pool(name="sp", bufs=sbufs))
        xpool = ctx.enter_context(tc.tile_pool(name="xps", bufs=1, space="PSUM"))
        hpool = ctx.enter_context(tc.tile_pool(name="hps", bufs=1, space="PSUM"))

        def const_col(val, tag):
            t = pool.tile([P, 1], dt.float32, tag=tag)
            nc.vector.memset(t[:], val)
            return t

        exp_bias = const_col(EXP_BIAS, "cbias_exp")
        zero_bias = const_col(0.0, "cbias_zero")

        # ---- input DMA ----
        f0 = pool.tile([P, FD], dt.float32, tag="scr", bufs=4, name="f0")
        nc.sync.dma_start(f0[:], f0_d.ap())
        lt = pool.tile([P, P], dt.float32)
        nc.gpsimd.dma_start(lt[:], lt_d.ap())
        diag = pool.tile([P, H, P], dt.float16)
        nc.gpsimd.dma_start(diag[:], diag_d.ap().rearrange("g p m -> p g m"))
        loud = pool.tile([P, FD], dt.float32, tag="loud")
        nc.scalar.dma_start(loud[:], loud_d.ap())
        mix = pool.tile([P, FD], dt.float32, tag="mix")
        nc.scalar.dma_start(mix[:], mix_d.ap())
        noise = pool.tile([P, FD], dt.float32, tag="noise")
        nc.scalar.dma_start(noise[:], noise_d.ap())

        # ---- stage 1: phase accumulation (turns) ----
        inc = pool.tile([P, FD], dt.float32, tag="scr", bufs=4, name="inc")
        nc.scalar.activation(inc[:], f0[:], AF.Exp, bias=exp_bias[:, 0:1], scale=1.0)
        local = pool.tile([P, FD], dt.float32, tag="scr", bufs=4, name="local")
        nc.vector.tensor_tensor_scan(local[:], inc[:], inc[:], 0.0,
                                     ALU.add, ALU.bypass)
        offs_ps = xpool.tile([P, 1], dt.float32, tag="x")
        nc.tensor.matmul(offs_ps[:], lt[:], local[:, FD - 1:FD],
                         start=True, stop=True)
        offs = pool.tile([P, 1], dt.float32)
        nc.vector.tensor_copy(offs[:], offs_ps[:])
        u = pool.tile([P, FD], dt.float32, tag="scr", bufs=4, name="u")
        nc.vector.tensor_scalar(u[:], local[:], offs[:, 0:1], None, ALU.add)
        ur = pool.tile([P, FD], dt.float32, tag="scr", bufs=4, name="ur")
        nc.vector.tensor_scalar(ur[:], u[:], MAGIC, MAGIC, ALU.add, ALU.subtract)
        u1 = pool.tile([P, FD], dt.float32, tag="u1")
        nc.vector.tensor_tensor(u1[:], u[:], ur[:], ALU.subtract)

        # ---- stage 2: harmonic bank, flat layout ----
        hw_ps = hpool.tile([P, 4, 512], dt.float32, tag="hw")
        chunks = []
        c0 = 0
        while c0 < FD:
            cn = min(512, FD - c0)
            chunks.append((c0, cn))
            c0 += cn

        order = _chains()
        vmap = {}
        v2_res = pool.tile([P, FD], dt.int32, tag="v2res")
        prev_odd = [None]

        Bm = pool.tile([P, FD], dt.float32, tag="Bm")
        Am = pool.tile([P, FD], dt.float32, tag="Am")
        A = pool.tile([P, FD], dt.float32, tag="A")
        epi_at = {order[min(len(order) - 1, 8)]: 0}
        emitted_epi = [False]

        def emit_epi():
            nc.gpsimd.tensor_tensor(Bm[:], loud[:], mix[:], ALU.mult)
            nc.gpsimd.tensor_tensor(Am[:], loud[:], Bm[:], ALU.subtract)
            nc.gpsimd.tensor_tensor(A[:], noise[:], Am[:], ALU.mult)
            emitted_epi[0] = True

        first_k = order[0]
        last_k = order[-1]
        for ki, k in enumerate(order):
            if k in epi_at and not emitted_epi[0]:
                emit_epi()
            if k == 1:
                v = vpool.tile([P, FD], dt.int32, tag="v")
                nc.vector.tensor_scalar(v[:], u1[:], Q32, None, ALU.mult)
            elif k % 2 == 0:
                src = vmap[k // 2]
                if k == 2:
                    v = v2_res
                else:
                    v = vpool.tile([P, FD], dt.int32, tag="v")
                nc.vector.tensor_scalar(v[:], src[:], 1, None,
                                        ALU.arith_shift_left)
            elif k in gp_odd and prev_odd[0] is not None:
                v = vpool.tile([P, FD], dt.int32, tag="v")
                nc.gpsimd.tensor_tensor(v[:], prev_odd[0][:], v2_res[:], ALU.add)
            else:
                w = vpool.tile([P, FD], dt.int32, tag="v")
                nc.vector.tensor_scalar(w[:], u1[:], float(k) * Q26, None,
                                        ALU.mult)
                v = vpool.tile([P, FD], dt.int32, tag="v")
                nc.vector.tensor_scalar(v[:], w[:], 6, None,
                                        ALU.arith_shift_left)
            vmap[k] = v
            if k % 2 == 1:
                prev_odd[0] = v

            s = spool.tile([P, FD], dt.float16, tag="s")
            nc.scalar.activation(s[:], v[:], AF.Sin, bias=zero_bias[:, 0:1],
                                 scale=float(2.0 * PI / Q32))

            for q, (c0, cn) in enumerate(chunks):
                nc.tensor.matmul(hw_ps[:, q, 0:cn], diag[:, k - 1, :],
                                 s[:, c0:c0 + cn],
                                 start=(k == first_k), stop=(k == last_k))

        if not emitted_epi[0]:
            emit_epi()

        # ---- epilogue: audio = Bm*hw + A, then peak-normalize per row ----
        hw_flat = hw_ps[:].rearrange("p q f -> p (q f)")[:, 0:FD]
        t1 = pool.tile([P, FD], dt.float32, tag="t1")
        nc.vector.tensor_tensor(t1[:], hw_flat, Bm[:], ALU.mult)
        audio = pool.tile([P, FD], dt.float32, tag="audio")
        nc.vector.tensor_tensor(audio[:], t1[:], A[:], ALU.add)

        pk = pool.tile([P, 1], dt.float32, tag="pk")
        nc.vector.tensor_reduce(pk[:], audio[:], axis=mybir.AxisListType.X,
                                op=ALU.max, apply_absolute_value=True)
        pkr = pool.tile([P, 32], dt.float32, tag="pkr")
        nc.vector.tensor_copy(pkr[:], pk[:, 0:1].to_broadcast((P, 32)))
        pkt = pool.tile([P, 32], dt.float32, tag="pkt")
        nc.vector.transpose(pkt[:], pkr[:])
        rowmax = pool.tile([P, 1], dt.float32, tag="rowmax")
        nc.vector.tensor_reduce(rowmax[:], pkt[:],
                                axis=mybir.AxisListType.X, op=ALU.max)
        pke = pool.tile([P, 1], dt.float32, tag="pke")
        nc.vector.tensor_scalar(pke[:], rowmax[:], 1e-6, None, ALU.add)
        rcp = pool.tile([P, 1], dt.float32, tag="rcp")
        nc.vector.reciprocal(rcp[:], pke[:])
        outt = pool.tile([P, FD], dt.float32, tag="outt")
        nc.vector.tensor_scalar(outt[:], audio[:], rcp[:, 0:1], None, ALU.mult)
        nc.sync.dma_start(out_d.ap(), outt[:])

    nc.compile()
    return nc


def kernel(f0, loudness, harmonic_mix, noise):
    if "nc" not in _cache:
        _cache["nc"] = _build()
        _cache["consts"] = _consts()
    nc = _cache["nc"]
    consts = _cache["consts"]

    def shard(a, c):
        return np.ascontiguousarray(
            a[c * RPC:(c + 1) * RPC].astype(f32, copy=False).reshape(P, FD))

    in_maps = []
    for c in range(NCORES):
        in_maps.append({
            "f0": shard(f0, c),
            "loud": shard(loudness, c),
            "mix": shard(harmonic_mix, c),
            "noise": shard(noise, c),
            **consts,
        })

    res = bass_utils.run_bass_kernel_spmd(nc, in_maps, core_ids=list(range(NCORES)))
    outs = [res.results[c]["audio"].reshape(RPC, T) for c in range(NCORES)]
    return np.concatenate(outs, axis=0)


# revision 18
# speedup vs baseline: 1.0978x; 1.0368x over previous
"""DDSP core synthesizer kernel for Trainium2 (8 NeuronCores, data-parallel).

Reference computation (per row of B=32, T=64000):
    f0_hz = 20*exp(f0); phase = cumsum(2*pi*f0_hz/SR)
    hw    = sum_k sin(phase*k)/k   (k = 1..60)
    audio = mix*hw*loud + (1-mix)*noise*loud;  out = audio / (max|audio| + 1e-6)

Device algorithm (phase in "turns"; harmonics in Q32 int fixed-point):
    inc  = exp(f0 + ln(20/SR))                       [ACT Exp]
    u    = blocked cumsum of inc                      [DVE scan + PE triangular mm]
    u1   = u - rint(u)  in [-0.5, 0.5]                [DVE magic rint + tt subtract]
    per harmonic k (flat layout: 128 blocks x 2000):
        v_k = k*u1*2^32 mod 2^32  (int32, exact wrap-around phase)
          k=1:    v_1 = int32(u1 * 2^32)              [DVE mult]
          even:   v_2m = v_m << 1                     [DVE arith_shift_left, wraps]
          odd:    w = int32(u1 * k*2^26); v = w << 6  [DVE mult + shift]
            or    v_k = v_{k-2} + v_2 (mod 2^32)      [GpSimd tt add, wraps]
        s_k = sin(2*pi*2^-32 * v_k)  -> f16           [ACT Sin, int32 input]
        hw += diag(1/k) @ s_k                         [PE f16 matmul, PSUM accum]
    epilogue: audio = Bm*hw + A with Bm = loud*mix, A = noise*(loud-Bm)
              peak-normalize per row (free abs-max + 32x32 transpose trick).

Sharding: pure data parallel, 4 rows per core, SPMD on cores 0-7.
"""

import sys

sys.path.insert(0, "/opt/trn_rl_repo")

import numpy as np
import ml_dtypes
from contextlib import ExitStack

import concourse.bass as bass
import concourse.tile as tile
from concourse import bacc, mybir
from concourse import bass_utils

f32 = np.float32
dt = mybir.dt

SR = 44100.0
H = 60
B, T = 32, 64000
NCORES = 8
RPC = B // NCORES           # rows per core = 4
P = 128
FD = T * RPC // P           # 2000
BPR = P // RPC              # 32
PI = float(np.pi)
MAGIC = float(1.5 * 2.0 ** 23)
EXP_BIAS = float(np.log(20.0 / SR))
Q26 = float(2.0 ** 26)
Q32 = float(2.0 ** 32)

_cache = {}


def _chains():
    order = []
    for h in range(1, H + 1, 2):
        k = h
        while k <= H:
            order.append(k)
            k *= 2
    return order


def _consts():
    kk, mm_ = np.meshgrid(np.arange(P), np.arange(P), indexing="ij")
    lt = ((kk // BPR == mm_ // BPR) & (kk % BPR < mm_ % BPR)).astype(f32)
    diag = np.zeros((H, P, P), dtype=np.float16)
    for k in range(1, H + 1):
        diag[k - 1, np.arange(P), np.arange(P)] = np.float16(1.0 / k)
    return {"lt": lt, "diag": diag}


def _build(gp_heads=9, vbufs=6, sbufs=5):
    nc = bacc.Bacc("TRN2", target_bir_lowering=False, debug=False,
                   enable_asserts=True, num_devices=NCORES)

    f0_d = nc.dram_tensor("f0", [P, FD], dt.float32, kind="ExternalInput")
    loud_d = nc.dram_tensor("loud", [P, FD], dt.float32, kind="ExternalInput")
    mix_d = nc.dram_tensor("mix", [P, FD], dt.float32, kind="ExternalInput")
    noise_d = nc.dram_tensor("noise", [P, FD], dt.float32, kind="ExternalInput")
    lt_d = nc.dram_tensor("lt", [P, P], dt.float32, kind="ExternalInput")
    diag_d = nc.dram_tensor("diag", [H, P, P], dt.float16, kind="ExternalInput")
    out_d = nc.dram_tensor("audio", [P, FD], dt.float32, kind="ExternalOutput")

    AF = mybir.ActivationFunctionType
    ALU = mybir.AluOpType

    gp_odd = set(range(3, 3 + 2 * gp_heads, 2)) & set(range(3, H, 2))

    with tile.TileContext(nc) as tc, ExitStack() as ctx:
        pool = ctx.enter_context(tc.tile_pool(name="sb", bufs=1))
        vpool = ctx.enter_context(tc.tile_pool(name="vp", bufs=vbufs))
        spool = ctx.enter_context(tc.tile_pool(name="sp", bufs=sbufs))
        xpool = ctx.enter_context(tc.tile_pool(name="xps", bufs=1, space="PSUM"))
        hpool = ctx.enter_context(tc.tile_pool(name="hps", bufs=1, space="PSUM"))

        def const_col(val, tag):
            t = pool.tile([P, 1], dt.float32, tag=tag)
            nc.vector.memset(t[:], val)
            return t

        exp_bias = const_col(EXP_BIAS, "cbias_exp")
        zero_bias = const_col(0.0, "cbias_zero")

        # ---- input DMA ----
        f0 = pool.tile([P, FD], dt.float32, tag="scr", bufs=4, name="f0")
        nc.sync.dma_start(f0[:], f0_d.ap())
        lt = pool.tile([P, P], dt.float32)
        nc.gpsimd.dma_start(lt[:], lt_d.ap())
        diag = pool.tile([P, H, P], dt.float16)
        nc.gpsimd.dma_start(diag[:], diag_d.ap().rearrange("g p m -> p g m"))
        loud = pool.tile([P, FD], dt.float32, tag="loud")
        nc.scalar.dma_start(loud[:], loud_d.ap())
        mix = pool.tile([P, FD], dt.float32, tag="mix")
        nc.scalar.dma_start(mix[:], mix_d.ap())
        noise = pool.tile([P, FD], dt.float32, tag="noise")
        nc.scalar.dma_start(noise[:], noise_d.ap())

        # ---- stage 1: phase accumulation (turns) ----
        inc = pool.tile([P, FD], dt.float32, tag="scr", bufs=4, name="inc")
        nc.scalar.activation(inc[:], f0[:], AF.Exp, bias=exp_bias[:, 0:1], scale=1.0)
        local = pool.tile([P, FD], dt.float32, tag="scr", bufs=4, name="local")
        nc.vector.tensor_tensor_scan(local[:], inc[:], inc[:], 0.0,
                                     ALU.add, ALU.bypass)
        offs_ps = xpool.tile([P, 1], dt.float32, tag="x")
        nc.tensor.matmul(offs_ps[:], lt[:], local[:, FD - 1:FD],
                         start=True, stop=True)
        offs = pool.tile([P, 1], dt.float32)
        nc.vector.tensor_copy(offs[:], offs_ps[:])
        u = pool.tile([P, FD], dt.float32, tag="scr", bufs=4, name="u")
        nc.vector.tensor_scalar(u[:], local[:], offs[:, 0:1], None, ALU.add)
        ur = pool.tile([P, FD], dt.float32, tag="scr", bufs=4, name="ur")
        nc.vector.tensor_scalar(ur[:], u[:], MAGIC, MAGIC, ALU.add, ALU.subtract)
        u1 = pool.tile([P, FD], dt.float32, tag="u1")
        nc.vector.tensor_tensor(u1[:], u[:], ur[:], ALU.subtract)

        # ---- stage 2: harmonic bank, flat layout ----
        hw_ps = hpool.tile([P, 4, 512], dt.float32, tag="hw")
        chunks = []
        c0 = 0
        while c0 < FD:
            cn = min(512, FD - c0)
            chunks.append((c0, cn))
            c0 += cn

        order = _chains()
        vmap = {}
        v2_res = pool.tile([P, FD], dt.int32, tag="v2res")
        prev_odd = [None]

        Bm = pool.tile([P, FD], dt.float32, tag="Bm")
        Am = pool.tile([P, FD], dt.float32, tag="Am")
        A = pool.tile([P, FD], dt.float32, tag="A")
        epi_at = {order[min(len(order) - 1, 8)]: 0}
        emitted_epi = [False]

        def emit_epi():
            nc.gpsimd.tensor_tensor(Bm[:], loud[:], mix[:], ALU.mult)
            nc.gpsimd.tensor_tensor(Am[:], loud[:], Bm[:], ALU.subtract)
            nc.gpsimd.tensor_tensor(A[:], noise[:], Am[:], ALU.mult)
            emitted_epi[0] = True

        first_k = order[0]
        last_k = order[-1]
        for ki, k in enumerate(order):
            if k in epi_at and not emitted_epi[0]:
                emit_epi()
            if k == 1:
                v = vpool.tile([P, FD], dt.int32, tag="v")
                nc.vector.tensor_scalar(v[:], u1[:], Q32, None, ALU.mult)
            elif k % 2 == 0:
                src = vmap[k // 2]
                if k == 2:
                    v = v2_res
                else:
                    v = vpool.tile([P, FD], dt.int32, tag="v")
                nc.vector.tensor_scalar(v[:], src[:], 1, None,
                                        ALU.arith_shift_left)
            elif k in gp_odd and prev_odd[0] is not None:
                v = vpool.tile([P, FD], dt.int32, tag="v")
                nc.gpsimd.tensor_tensor(v[:], prev_odd[0][:], v2_res[:], ALU.add)
            else:
                w = vpool.tile([P, FD], dt.int32, tag="v")
                nc.vector.tensor_scalar(w[:], u1[:], float(k) * Q26, None,
                                        ALU.mult)
                v = vpool.tile([P, FD], dt.int32, tag="v")
                nc.vector.tensor_scalar(v[:], w[:], 6, None,
                                        ALU.arith_shift_left)
            vmap[k] = v
            if k % 2 == 1:
                prev_odd[0] = v

            s = spool.tile([P, FD], dt.float16, tag="s")
            nc.scalar.activation(s[:], v[:], AF.Sin, bias=zero_bias[:, 0:1],
                                 scale=float(2.0 * PI / Q32))

            for q, (c0, cn) in enumerate(chunks):
                nc.tensor.matmul(hw_ps[:, q, 0:cn], diag[:, k - 1, :],
                                 s[:, c0:c0 + cn],
                                 start=(k == first_k), stop=(k == last_k))

        if not emitted_epi[0]:
            emit_epi()

        # ---- epilogue: audio = Bm*hw + A, then peak-normalize per row ----
        hw_flat = hw_ps[:].rearrange("p q f -> p (q f)")[:, 0:FD]
        t1 = pool.tile([P, FD], dt.float32, tag="t1")
        nc.vector.tensor_tensor(t1[:], hw_flat, Bm[:], ALU.mult)
        audio = pool.tile([P, FD], dt.float32, tag="audio")
        nc.vector.tensor_tensor(audio[:], t1[:], A[:], ALU.add)

        pk = pool.tile([P, 1], dt.float32, tag="pk")
        nc.vector.tensor_reduce(pk[:], audio[:], axis=mybir.AxisListType.X,
                                op=ALU.max, apply_absolute_value=True)
        pkr = pool.tile([P, 32], dt.float32, tag="pkr")
        nc.vector.tensor_copy(pkr[:], pk[:, 0:1].to_broadcast((P, 32)))
        pkt = pool.tile([P, 32], dt.float32, tag="pkt")
        nc.vector.transpose(pkt[:], pkr[:])
        rowmax = pool.tile([P, 1], dt.float32, tag="rowmax")
        nc.vector.tensor_reduce(rowmax[:], pkt[:],
                                axis=mybir.AxisListType.X, op=ALU.max)
        pke = pool.tile([P, 1], dt.float32, tag="pke")
        nc.vector.tensor_scalar(pke[:], rowmax[:], 1e-6, None, ALU.add)
        rcp = pool.tile([P, 1], dt.float32, tag="rcp")
        nc.vector.reciprocal(rcp[:], pke[:])
        outt = pool.tile([P, FD], dt.float32, tag="outt")
        nc.vector.tensor_scalar(outt[:], audio[:], rcp[:, 0:1], None, ALU.mult)
        nc.sync.dma_start(out_d.ap(), outt[:])

    nc.compile()
    return nc


def kernel(f0, loudness, harmonic_mix, noise):
    if "nc" not in _cache:
        _cache["nc"] = _build()
        _cache["consts"] = _consts()
    nc = _cache["nc"]
    consts = _cache["consts"]

    def shard(a, c):
        return np.ascontiguousarray(
            a[c * RPC:(c + 1) * RPC].astype(f32, copy=False).reshape(P, FD))

    in_maps = []
    for c in range(NCORES):
        in_maps.append({
            "f0": shard(f0, c),
            "loud": shard(loudness, c),
            "mix": shard(harmonic_mix, c),
            "noise": shard(noise, c),
            **consts,
        })

    res = bass_utils.run_bass_kernel_spmd(nc, in_maps, core_ids=list(range(NCORES)))
    outs = [res.results[c]["audio"].reshape(RPC, T) for c in range(NCORES)]
    return np.concatenate(outs, axis=0)


# revision 19
# speedup vs baseline: 1.1235x; 1.0234x over previous
"""DDSP core synthesizer kernel for Trainium2 (8 NeuronCores, data-parallel).

Reference computation (per row of B=32, T=64000):
    f0_hz = 20*exp(f0); phase = cumsum(2*pi*f0_hz/SR)
    hw    = sum_k sin(phase*k)/k   (k = 1..60)
    audio = mix*hw*loud + (1-mix)*noise*loud;  out = audio / (max|audio| + 1e-6)

Device algorithm (phase in "turns"; harmonics in Q32 int fixed-point):
    inc  = exp(f0 + ln(20/SR))                       [ACT Exp]
    u    = blocked cumsum of inc                      [DVE scan + PE triangular mm]
    u1   = u - rint(u)  in [-0.5, 0.5]                [DVE magic rint + tt subtract]
    per harmonic k (flat layout: 128 blocks x 2000):
        v_k = k*u1*2^32 mod 2^32  (int32, exact wrap-around phase)
          k=1:    v_1 = int32(u1 * 2^32)              [DVE mult]
          even:   v_2m = v_m << 1                     [DVE arith_shift_left, wraps]
          odd:    w = int32(u1 * k*2^26); v = w << 6  [DVE mult + shift]
            or    v_k = v_{k-2} + v_2 (mod 2^32)      [GpSimd tt add, wraps]
        s_k = sin(2*pi*2^-32 * v_k)  -> f16           [ACT Sin, int32 input]
        hw += diag(1/k) @ s_k                         [PE f16 matmul, PSUM accum]
    epilogue: audio = Bm*hw + A with Bm = loud*mix, A = noise*(loud-Bm)
              peak-normalize per row (free abs-max + 32x32 transpose trick).

Sharding: pure data parallel, 4 rows per core, SPMD on cores 0-7.
"""

import sys

sys.path.insert(0, "/opt/trn_rl_repo")

import numpy as np
import ml_dtypes
from contextlib import ExitStack

import concourse.bass as bass
import concourse.tile as tile
from concourse import bacc, mybir
from concourse import bass_utils

f32 = np.float32
dt = mybir.dt

SR = 44100.0
H = 60
B, T = 32, 64000
NCORES = 8
RPC = B // NCORES           # rows per core = 4
P = 128
FD = T * RPC // P           # 2000
BPR = P // RPC              # 32
PI = float(np.pi)
MAGIC = float(1.5 * 2.0 ** 23)
EXP_BIAS = float(np.log(20.0 / SR))
Q26 = float(2.0 ** 26)
Q32 = float(2.0 ** 32)

_cache = {}


def _chains():
    order = []
    for h in range(1, H + 1, 2):
        k = h
        while k <= H:
            order.append(k)
            k *= 2
    return order


def _consts():
    kk, mm_ = np.meshgrid(np.arange(P), np.arange(P), indexing="ij")
    lt = ((kk // BPR == mm_ // BPR) & (kk % BPR < mm_ % BPR)).astype(f32)
    diag = np.zeros((H, P, P), dtype=np.float16)
    for k in range(1, H + 1):
        diag[k - 1, np.arange(P), np.arange(P)] = np.float16(1.0 / k)
    return {"lt": lt, "diag": diag}


def _build(gp_heads=6, vbufs=6, sbufs=5):
    nc = bacc.Bacc("TRN2", target_bir_lowering=False, debug=False,
                   enable_asserts=True, num_devices=NCORES)

    f0_d = nc.dram_tensor("f0", [P, FD], dt.float32, kind="ExternalInput")
    loud_d = nc.dram_tensor("loud", [P, FD], dt.float32, kind="ExternalInput")
    mix_d = nc.dram_tensor("mix", [P, FD], dt.float32, kind="ExternalInput")
    noise_d = nc.dram_tensor("noise", [P, FD], dt.float32, kind="ExternalInput")
    lt_d = nc.dram_tensor("lt", [P, P], dt.float32, kind="ExternalInput")
    diag_d = nc.dram_tensor("diag", [H, P, P], dt.float16, kind="ExternalInput")
    out_d = nc.dram_tensor("audio", [P, FD], dt.float32, kind="ExternalOutput")

    AF = mybir.ActivationFunctionType
    ALU = mybir.AluOpType

    gp_odd = set(range(3, 3 + 2 * gp_heads, 2)) & set(range(3, H, 2))

    with tile.TileContext(nc) as tc, ExitStack() as ctx:
        pool = ctx.enter_context(tc.tile_pool(name="sb", bufs=1))
        vpool = ctx.enter_context(tc.tile_pool(name="vp", bufs=vbufs))
        spool = ctx.enter_context(tc.tile_pool(name="sp", bufs=sbufs))
        xpool = ctx.enter_context(tc.tile_pool(name="xps", bufs=1, space="PSUM"))
        hpool = ctx.enter_context(tc.tile_pool(name="hps", bufs=1, space="PSUM"))

        def const_col(val, tag):
            t = pool.tile([P, 1], dt.float32, tag=tag)
            nc.vector.memset(t[:], val)
            return t

        exp_bias = const_col(EXP_BIAS, "cbias_exp")
        zero_bias = const_col(0.0, "cbias_zero")

        # ---- input DMA ----
        f0 = pool.tile([P, FD], dt.float32, tag="scr", bufs=4, name="f0")
        nc.sync.dma_start(f0[:], f0_d.ap())
        lt = pool.tile([P, P], dt.float32)
        nc.gpsimd.dma_start(lt[:], lt_d.ap())
        diag = pool.tile([P, H, P], dt.float16)
        nc.gpsimd.dma_start(diag[:], diag_d.ap().rearrange("g p m -> p g m"))
        loud = pool.tile([P, FD], dt.float32, tag="loud")
        nc.scalar.dma_start(loud[:], loud_d.ap())
        mix = pool.tile([P, FD], dt.float32, tag="mix")
        nc.scalar.dma_start(mix[:], mix_d.ap())
        noise = pool.tile([P, FD], dt.float32, tag="noise")
        nc.scalar.dma_start(noise[:], noise_d.ap())

        # ---- stage 1: phase accumulation (turns) ----
        inc = pool.tile([P, FD], dt.float32, tag="scr", bufs=4, name="inc")
        nc.scalar.activation(inc[:], f0[:], AF.Exp, bias=exp_bias[:, 0:1], scale=1.0)
        local = pool.tile([P, FD], dt.float32, tag="scr", bufs=4, name="local")
        nc.vector.tensor_tensor_scan(local[:], inc[:], inc[:], 0.0,
                                     ALU.add, ALU.bypass)
        offs_ps = xpool.tile([P, 1], dt.float32, tag="x")
        nc.tensor.matmul(offs_ps[:], lt[:], local[:, FD - 1:FD],
                         start=True, stop=True)
        offs = pool.tile([P, 1], dt.float32)
        nc.vector.tensor_copy(offs[:], offs_ps[:])
        u = pool.tile([P, FD], dt.float32, tag="scr", bufs=4, name="u")
        nc.vector.tensor_scalar(u[:], local[:], offs[:, 0:1], None, ALU.add)
        ur = pool.tile([P, FD], dt.float32, tag="scr", bufs=4, name="ur")
        nc.vector.tensor_scalar(ur[:], u[:], MAGIC, MAGIC, ALU.add, ALU.subtract)
        u1 = pool.tile([P, FD], dt.float32, tag="u1")
        nc.vector.tensor_tensor(u1[:], u[:], ur[:], ALU.subtract)

        # ---- stage 2: harmonic bank, flat layout ----
        hw_ps = hpool.tile([P, 4, 512], dt.float32, tag="hw")
        chunks = []
        c0 = 0
        while c0 < FD:
            cn = min(512, FD - c0)
            chunks.append((c0, cn))
            c0 += cn

        order = _chains()
        vmap = {}
        v2_res = pool.tile([P, FD], dt.int32, tag="v2res")
        prev_odd = [None]

        Bm = pool.tile([P, FD], dt.float32, tag="Bm")
        Am = pool.tile([P, FD], dt.float32, tag="Am")
        A = pool.tile([P, FD], dt.float32, tag="A")
        epi_at = {order[min(len(order) - 1, 8)]: 0}
        emitted_epi = [False]

        def emit_epi():
            nc.gpsimd.tensor_tensor(Bm[:], loud[:], mix[:], ALU.mult)
            nc.gpsimd.tensor_tensor(Am[:], loud[:], Bm[:], ALU.subtract)
            nc.gpsimd.tensor_tensor(A[:], noise[:], Am[:], ALU.mult)
            emitted_epi[0] = True

        first_k = order[0]
        last_k = order[-1]
        for ki, k in enumerate(order):
            if k in epi_at and not emitted_epi[0]:
                emit_epi()
            if k == 1:
                v = vpool.tile([P, FD], dt.int32, tag="v")
                nc.vector.tensor_scalar(v[:], u1[:], Q32, None, ALU.mult)
            elif k % 2 == 0:
                src = vmap[k // 2]
                if k == 2:
                    v = v2_res
                else:
                    v = vpool.tile([P, FD], dt.int32, tag="v")
                nc.vector.tensor_scalar(v[:], src[:], 1, None,
                                        ALU.arith_shift_left)
            elif k in gp_odd and prev_odd[0] is not None:
                v = vpool.tile([P, FD], dt.int32, tag="v")
                nc.gpsimd.tensor_tensor(v[:], prev_odd[0][:], v2_res[:], ALU.add)
            else:
                w = vpool.tile([P, FD], dt.int32, tag="v")
                nc.vector.tensor_scalar(w[:], u1[:], float(k) * Q26, None,
                                        ALU.mult)
                v = vpool.tile([P, FD], dt.int32, tag="v")
                nc.vector.tensor_scalar(v[:], w[:], 6, None,
                                        ALU.arith_shift_left)
            vmap[k] = v
            if k % 2 == 1:
                prev_odd[0] = v

            s = spool.tile([P, FD], dt.float16, tag="s")
            nc.scalar.activation(s[:], v[:], AF.Sin, bias=zero_bias[:, 0:1],
                                 scale=float(2.0 * PI / Q32))

            for q, (c0, cn) in enumerate(chunks):
                nc.tensor.matmul(hw_ps[:, q, 0:cn], diag[:, k - 1, :],
                                 s[:, c0:c0 + cn],
                                 start=(k == first_k), stop=(k == last_k))

        if not emitted_epi[0]:
            emit_epi()

        # ---- epilogue: audio = Bm*hw + A, then peak-normalize per row ----
        hw_flat = hw_ps[:].rearrange("p q f -> p (q f)")[:, 0:FD]
        t1 = pool.tile([P, FD], dt.float32, tag="t1")
        nc.vector.tensor_tensor(t1[:], hw_flat, Bm[:], ALU.mult)
        audio = pool.tile([P, FD], dt.float32, tag="audio")
        nc.vector.tensor_tensor(audio[:], t1[:], A[:], ALU.add)

        pk = pool.tile([P, 1], dt.float32, tag="pk")
        nc.vector.tensor_reduce(pk[:], audio[:], axis=mybir.AxisListType.X,
                                op=ALU.max, apply_absolute_value=True)
        pkr = pool.tile([P, 32], dt.float32, tag="pkr")
        nc.vector.tensor_copy(pkr[:], pk[:, 0:1].to_broadcast((P, 32)))
        pkt = pool.tile([P, 32], dt.float32, tag="pkt")
        nc.vector.transpose(pkt[:], pkr[:])
        rowmax = pool.tile([P, 1], dt.float32, tag="rowmax")
        nc.vector.tensor_reduce(rowmax[:], pkt[:],
                                axis=mybir.AxisListType.X, op=ALU.max)
        pke = pool.tile([P, 1], dt.float32, tag="pke")
        nc.vector.tensor_scalar(pke[:], rowmax[:], 1e-6, None, ALU.add)
        rcp = pool.tile([P, 1], dt.float32, tag="rcp")
        nc.vector.reciprocal(rcp[:], pke[:])
        outt = pool.tile([P, FD], dt.float32, tag="outt")
        nc.vector.tensor_scalar(outt[:], audio[:], rcp[:, 0:1], None, ALU.mult)
        nc.sync.dma_start(out_d.ap(), outt[:])

    nc.compile()
    return nc


def kernel(f0, loudness, harmonic_mix, noise):
    if "nc" not in _cache:
        _cache["nc"] = _build()
        _cache["consts"] = _consts()
    nc = _cache["nc"]
    consts = _cache["consts"]

    def shard(a, c):
        return np.ascontiguousarray(
            a[c * RPC:(c + 1) * RPC].astype(f32, copy=False).reshape(P, FD))

    in_maps = []
    for c in range(NCORES):
        in_maps.append({
            "f0": shard(f0, c),
            "loud": shard(loudness, c),
            "mix": shard(harmonic_mix, c),
            "noise": shard(noise, c),
            **consts,
        })

    res = bass_utils.run_bass_kernel_spmd(nc, in_maps, core_ids=list(range(NCORES)))
    outs = [res.results[c]["audio"].reshape(RPC, T) for c in range(NCORES)]
    return np.concatenate(outs, axis=0)


# revision 20
# speedup vs baseline: 1.1698x; 1.0412x over previous
"""DDSP core synthesizer kernel for Trainium2 (8 NeuronCores, data-parallel).

Reference computation (per row of B=32, T=64000):
    f0_hz = 20*exp(f0); phase = cumsum(2*pi*f0_hz/SR)
    hw    = sum_k sin(phase*k)/k   (k = 1..60)
    audio = mix*hw*loud + (1-mix)*noise*loud;  out = audio / (max|audio| + 1e-6)

Device algorithm (phase in "turns"; harmonics in Q32 int fixed-point):
    inc  = exp(f0 + ln(20/SR))                       [ACT Exp]
    u    = blocked cumsum of inc                      [DVE scan + PE triangular mm]
    u1   = u - rint(u)  in [-0.5, 0.5]                [DVE magic rint + tt subtract]
    per harmonic k (flat layout: 128 blocks x 2000):
        v_k = k*u1*2^32 mod 2^32  (int32, exact wrap-around phase)
          k=1:    v_1 = int32(u1 * 2^32)              [DVE mult]
          even:   v_2m = v_m << 1                     [DVE arith_shift_left, wraps]
          odd:    w = int32(u1 * k*2^26); v = w << 6  [DVE mult + shift]
            or    v_k = v_{k-2} + v_2 (mod 2^32)      [GpSimd tt add, wraps]
        s_k = sin(2*pi*2^-32 * v_k)  -> f16           [ACT Sin, int32 input]
        hw += diag(1/k) @ s_k                         [PE f16 matmul, PSUM accum]
    epilogue: audio = Bm*hw + A with Bm = loud*mix, A = noise*(loud-Bm)
              peak-normalize per row (free abs-max + 32x32 transpose trick).

Sharding: pure data parallel, 4 rows per core, SPMD on cores 0-7.
"""

import sys

sys.path.insert(0, "/opt/trn_rl_repo")

import numpy as np
import ml_dtypes
from contextlib import ExitStack

import concourse.bass as bass
import concourse.tile as tile
from concourse import bacc, mybir
from concourse import bass_utils

f32 = np.float32
dt = mybir.dt

SR = 44100.0
H = 60
B, T = 32, 64000
NCORES = 8
RPC = B // NCORES           # rows per core = 4
P = 128
FD = T * RPC // P           # 2000
BPR = P // RPC              # 32
PI = float(np.pi)
MAGIC = float(1.5 * 2.0 ** 23)
EXP_BIAS = float(np.log(20.0 / SR))
Q26 = float(2.0 ** 26)
Q32 = float(2.0 ** 32)

_cache = {}


def _chains():
    order = []
    for h in range(1, H + 1, 2):
        k = h
        while k <= H:
            order.append(k)
            k *= 2
    return order


def _consts():
    kk, mm_ = np.meshgrid(np.arange(P), np.arange(P), indexing="ij")
    lt = ((kk // BPR == mm_ // BPR) & (kk % BPR < mm_ % BPR)).astype(f32)
    diag = np.zeros((H, P, P), dtype=np.float16)
    for k in range(1, H + 1):
        diag[k - 1, np.arange(P), np.arange(P)] = np.float16(1.0 / k)
    return {"lt": lt, "diag": diag}


def _build(gp_heads=0, vbufs=6, sbufs=5):
    nc = bacc.Bacc("TRN2", target_bir_lowering=False, debug=False,
                   enable_asserts=True, num_devices=NCORES)

    f0_d = nc.dram_tensor("f0", [P, FD], dt.float32, kind="ExternalInput")
    loud_d = nc.dram_tensor("loud", [P, FD], dt.float32, kind="ExternalInput")
    mix_d = nc.dram_tensor("mix", [P, FD], dt.float32, kind="ExternalInput")
    noise_d = nc.dram_tensor("noise", [P, FD], dt.float32, kind="ExternalInput")
    lt_d = nc.dram_tensor("lt", [P, P], dt.float32, kind="ExternalInput")
    diag_d = nc.dram_tensor("diag", [H, P, P], dt.float16, kind="ExternalInput")
    out_d = nc.dram_tensor("audio", [P, FD], dt.float32, kind="ExternalOutput")

    AF = mybir.ActivationFunctionType
    ALU = mybir.AluOpType

    gp_odd = set(range(3, 3 + 2 * gp_heads, 2)) & set(range(3, H, 2))

    with tile.TileContext(nc) as tc, ExitStack() as ctx:
        pool = ctx.enter_context(tc.tile_pool(name="sb", bufs=1))
        vpool = ctx.enter_context(tc.tile_pool(name="vp", bufs=vbufs))
        spool = ctx.enter_context(tc.tile_pool(name="sp", bufs=sbufs))
        xpool = ctx.enter_context(tc.tile_pool(name="xps", bufs=1, space="PSUM"))
        hpool = ctx.enter_context(tc.tile_pool(name="hps", bufs=1, space="PSUM"))

        def const_col(val, tag):
            t = pool.tile([P, 1], dt.float32, tag=tag)
            nc.vector.memset(t[:], val)
            return t

        exp_bias = const_col(EXP_BIAS, "cbias_exp")
        zero_bias = const_col(0.0, "cbias_zero")

        # ---- input DMA ----
        f0 = pool.tile([P, FD], dt.float32, tag="scr", bufs=4, name="f0")
        nc.sync.dma_start(f0[:], f0_d.ap())
        lt = pool.tile([P, P], dt.float32)
        nc.gpsimd.dma_start(lt[:], lt_d.ap())
        diag = pool.tile([P, H, P], dt.float16)
        nc.gpsimd.dma_start(diag[:], diag_d.ap().rearrange("g p m -> p g m"))
        loud = pool.tile([P, FD], dt.float32, tag="loud")
        nc.scalar.dma_start(loud[:], loud_d.ap())
        mix = pool.tile([P, FD], dt.float32, tag="mix")
        nc.scalar.dma_start(mix[:], mix_d.ap())
        noise = pool.tile([P, FD], dt.float32, tag="noise")
        nc.scalar.dma_start(noise[:], noise_d.ap())

        # ---- stage 1: phase accumulation (turns) ----
        inc = pool.tile([P, FD], dt.float32, tag="scr", bufs=4, name="inc")
        nc.scalar.activation(inc[:], f0[:], AF.Exp, bias=exp_bias[:, 0:1], scale=1.0)
        local = pool.tile([P, FD], dt.float32, tag="scr", bufs=4, name="local")
        nc.vector.tensor_tensor_scan(local[:], inc[:], inc[:], 0.0,
                                     ALU.add, ALU.bypass)
        offs_ps = xpool.tile([P, 1], dt.float32, tag="x")
        nc.tensor.matmul(offs_ps[:], lt[:], local[:, FD - 1:FD],
                         start=True, stop=True)
        offs = pool.tile([P, 1], dt.float32)
        nc.vector.tensor_copy(offs[:], offs_ps[:])
        u = pool.tile([P, FD], dt.float32, tag="scr", bufs=4, name="u")
        nc.vector.tensor_scalar(u[:], local[:], offs[:, 0:1], None, ALU.add)
        ur = pool.tile([P, FD], dt.float32, tag="scr", bufs=4, name="ur")
        nc.vector.tensor_scalar(ur[:], u[:], MAGIC, MAGIC, ALU.add, ALU.subtract)
        u1 = pool.tile([P, FD], dt.float32, tag="u1")
        nc.vector.tensor_tensor(u1[:], u[:], ur[:], ALU.subtract)

        # ---- stage 2: harmonic bank, flat layout ----
        hw_ps = hpool.tile([P, 4, 512], dt.float32, tag="hw")
        chunks = []
        c0 = 0
        while c0 < FD:
            cn = min(512, FD - c0)
            chunks.append((c0, cn))
            c0 += cn

        order = _chains()
        vmap = {}
        v2_res = pool.tile([P, FD], dt.int32, tag="v2res")
        prev_odd = [None]

        Bm = pool.tile([P, FD], dt.float32, tag="Bm")
        Am = pool.tile([P, FD], dt.float32, tag="Am")
        A = pool.tile([P, FD], dt.float32, tag="A")
        epi_at = {order[min(len(order) - 1, 8)]: 0}
        emitted_epi = [False]

        def emit_epi():
            nc.gpsimd.tensor_tensor(Bm[:], loud[:], mix[:], ALU.mult)
            nc.gpsimd.tensor_tensor(Am[:], loud[:], Bm[:], ALU.subtract)
            nc.gpsimd.tensor_tensor(A[:], noise[:], Am[:], ALU.mult)
            emitted_epi[0] = True

        first_k = order[0]
        last_k = order[-1]
        for ki, k in enumerate(order):
            if k in epi_at and not emitted_epi[0]:
                emit_epi()
            if k == 1:
                v = vpool.tile([P, FD], dt.int32, tag="v")
                nc.vector.tensor_scalar(v[:], u1[:], Q32, None, ALU.mult)
            elif k % 2 == 0:
                src = vmap[k // 2]
                if k == 2:
                    v = v2_res
                else:
                    v = vpool.tile([P, FD], dt.int32, tag="v")
                nc.vector.tensor_scalar(v[:], src[:], 1, None,
                                        ALU.arith_shift_left)
            elif k in gp_odd and prev_odd[0] is not None:
                v = vpool.tile([P, FD], dt.int32, tag="v")
                nc.gpsimd.tensor_tensor(v[:], prev_odd[0][:], v2_res[:], ALU.add)
            else:
                w = vpool.tile([P, FD], dt.int32, tag="v")
                nc.vector.tensor_scalar(w[:], u1[:], float(k) * Q26, None,
                                        ALU.mult)
                v = vpool.tile([P, FD], dt.int32, tag="v")
                nc.vector.tensor_scalar(v[:], w[:], 6, None,
                                        ALU.arith_shift_left)
            vmap[k] = v
            if k % 2 == 1:
                prev_odd[0] = v

            s = spool.tile([P, FD], dt.float16, tag="s")
            nc.scalar.activation(s[:], v[:], AF.Sin, bias=zero_bias[:, 0:1],
                                 scale=float(2.0 * PI / Q32))

            for q, (c0, cn) in enumerate(chunks):
                nc.tensor.matmul(hw_ps[:, q, 0:cn], diag[:, k - 1, :],
                                 s[:, c0:c0 + cn],
                                 start=(k == first_k), stop=(k == last_k))

        if not emitted_epi[0]:
            emit_epi()

        # ---- epilogue: audio = Bm*hw + A, then peak-normalize per row ----
        hw_flat = hw_ps[:].rearrange("p q f -> p (q f)")[:, 0:FD]
        t1 = pool.tile([P, FD], dt.float32, tag="t1")
        nc.vector.tensor_tensor(t1[:], hw_flat, Bm[:], ALU.mult)
        audio = pool.tile([P, FD], dt.float32, tag="audio")
        nc.vector.tensor_tensor(audio[:], t1[:], A[:], ALU.add)

        pk = pool.tile([P, 1], dt.float32, tag="pk")
        nc.vector.tensor_reduce(pk[:], audio[:], axis=mybir.AxisListType.X,
                                op=ALU.max, apply_absolute_value=True)
        pkr = pool.tile([P, 32], dt.float32, tag="pkr")
        nc.vector.tensor_copy(pkr[:], pk[:, 0:1].to_broadcast((P, 32)))
        pkt = pool.tile([P, 32], dt.float32, tag="pkt")
        nc.vector.transpose(pkt[:], pkr[:])
        rowmax = pool.tile([P, 1], dt.float32, tag="rowmax")
        nc.vector.tensor_reduce(rowmax[:], pkt[:],
                                axis=mybir.AxisListType.X, op=ALU.max)
        pke = pool.tile([P, 1], dt.float32, tag="pke")
        nc.vector.tensor_scalar(pke[:], rowmax[:], 1e-6, None, ALU.add)
        rcp = pool.tile([P, 1], dt.float32, tag="rcp")
        nc.vector.reciprocal(rcp[:], pke[:])
        outt = pool.tile([P, FD], dt.float32, tag="outt")
        nc.vector.tensor_scalar(outt[:], audio[:], rcp[:, 0:1], None, ALU.mult)
        nc.sync.dma_start(out_d.ap(), outt[:])

    nc.compile()
    return nc


def kernel(f0, loudness, harmonic_mix, noise):
    if "nc" not in _cache:
        _cache["nc"] = _build()
        _cache["consts"] = _consts()
    nc = _cache["nc"]
    consts = _cache["consts"]

    def shard(a, c):
        return np.ascontiguousarray(
            a[c * RPC:(c + 1) * RPC].astype(f32, copy=False).reshape(P, FD))

    in_maps = []
    for c in range(NCORES):
        in_maps.append({
            "f0": shard(f0, c),
            "loud": shard(loudness, c),
            "mix": shard(harmonic_mix, c),
            "noise": shard(noise, c),
            **consts,
        })

    res = bass_utils.run_bass_kernel_spmd(nc, in_maps, core_ids=list(range(NCORES)))
    outs = [res.results[c]["audio"].reshape(RPC, T) for c in range(NCORES)]
    return np.concatenate(outs, axis=0)
